# revision 4
# baseline (speedup 1.0000x reference)
"""Trainium2 Bass kernel for nn_Attention_50921132262075.

Reference computation (per batch b):
    q = Wq @ x_b    [32, 4096]      (1x1 conv == channel matmul)
    k = Wk @ y_b    [32, 4096]
    v = Wv @ y_b    [256, 4096]
    e[i, j] = q[:, i] . k[:, j]
    a = softmax_j(e)
    out[c, i] = sum_j v[c, j] a[i, j]
    result = gamma * out + x_b

Sharding: 8 cores = 4 batches x 2 query-halves. Each core gets the full
y of its batch (keys/values) plus a 2048-query slice of x, computes
q/k/v projections on chip, and runs flash-style attention over its
2048 queries x 4096 keys.

Device-side design (fp8 generation; ~154 us/core was the bf16 z-stream
version, ~213 us the session baseline):
  * Energy is computed TRANSPOSED (eT[j, i], keys on partitions) in
    fp32r so the probability tiles are already the [K=j, N=i] moving
    operand the z-matmuls need -- zero on-chip transposes.  QK has
    K=32 only, so the PE is row-tiled 4x via tile_position.
  * The z-stream runs in FP8 (e4m3) DoubleRow mode: two key-chunks per
    matmul (K=256), measured ~140ns vs ~200ns for the equivalent two
    bf16 matmuls.  p8 = pt * (T/l) with T=128 guarantees p8 <= ~130,
    safely below e4m3's 240 limit (the fp8 convert does NOT saturate:
    >=248 becomes inf).  The exact softmax denominator l comes from a
    cheap PE bf16 ones-matmul stream over the exp'd tiles (bf16
    matmuls stream 2 cols/cycle on HW -- 32 matmuls/chunk is only
    ~3.4us), which freed DVE+Pool from the old elementwise
    accumulation.  The fp8 denominator l8 = sum_j p8 (fp8 ones-matmul
    stream) makes fp8 rounding/flush self-consistent between
    numerator and denominator; measured end-to-end rel err ~1.2e-2
    against the fp32 reference (gate 2e-2).
  * The fp8 conversion pass (p8 = pt * rect) is the only remaining
    elementwise sweep; it is split DVE/Pool ~10:6 by measured rates
    (DVE 1.6ns/col with fp8 out, Pool 2.7ns/col).
  * ScalarE is the bottleneck engine (64 exps x ~1.5us); everything
    else (PE ~75us, DVE/Pool ~85us) hides under the exp stream, which
    paces the software pipeline: chunk ic's QK+exp run while chunk
    ic-1's convert+z8-streams and chunk ic-2's tail ride along.
  * exp runs as [128, 1024] psum->bf16 ops; q-path inputs load through
    the Activation HWDGE queue, k/y/yT8 through SP+Act queues.
"""

import ml_dtypes
import numpy as np

import concourse.bass as bass
import concourse.mybir as mybir
import concourse.tile as tile
from concourse.bass_utils import run_bass_kernel_spmd
from concourse.vector_clock import ScopedClock, VectorClock

# ---------------------------------------------------------------------------
# Workaround: this walrus build rejects instructions carrying more than one
# semaphore wait ("Too many sync wait commands" in setupSyncWait). Split
# multi-wait instructions into single-wait NoOps on the same engine (engines
# execute their stream in order, so semantics are unchanged), and emit the
# kernel-tail drain as one drain per proc instead of one drain with N waits.
# ---------------------------------------------------------------------------
_orig_commit = tile.TileContext._commit_instruction
_split_counter = [0]


def _commit_split_waits(self, inst, lazy_reg_writes: bool = True):
    si = getattr(inst, "sync_info", None)
    if si is not None and si.on_wait is not None and len(si.on_wait) > 1:
        waits = list(si.on_wait)
        for w in waits[:-1]:
            _split_counter[0] += 1
            nop = mybir.InstNoOp(
                name=f"{inst.name}-ws{_split_counter[0]}",
                engine=inst.engine,
                bass_nofuse=True,
                sync_info=mybir.SyncInfo(on_wait=[w], on_update=[]),
            )
            _orig_commit(self, nop, lazy_reg_writes)
        inst.sync_info = mybir.SyncInfo(
            on_wait=[waits[-1]], on_update=list(si.on_update or [])
        )
    return _orig_commit(self, inst, lazy_reg_writes)


def _split_drain_and_barrier(self, tick_clock, wait_clock):
    nc = self.nc
    gc = tick_clock.global_clock
    n = len(gc)
    for p in range(n):
        if gc[p] <= 0:
            continue
        partial = VectorClock([gc[q] if q == p else 0 for q in range(n)])
        d = nc.sync.drain()
        wait_clock.add_sem_waits(d.ins, ScopedClock({None: partial}))
    nc.all_engine_barrier()
    assert self.sems is not None
    popped = nc._tile_sem_poison_stack.pop()
    assert popped is self._sem_poison
    nc.clear_and_free_semaphores(list(self.sems.allocated().values()))
    nc.all_engine_barrier()


def _apply_walrus_workarounds():
    tile.TileContext._commit_instruction = _commit_split_waits
    tile.TileContext._drain_and_barrier = _split_drain_and_barrier


_apply_walrus_workarounds()

# ---------------------------------------------------------------------------
# Problem constants (hardcoded per the task contract).
# ---------------------------------------------------------------------------
B, C, C8 = 4, 256, 32
HW_N = 4096          # keys per batch (H*W)
NQ = 2048            # queries per core (half a batch)
P = 128
NCORES = 8
F32 = mybir.dt.float32
F32R = mybir.dt.float32r
BF16 = mybir.dt.bfloat16
FP8 = mybir.dt.float8e4
AF = mybir.ActivationFunctionType
DR = mybir.MatmulPerfMode.DoubleRow

T_SCALE = 128.0      # p8 = pt * T/l; max p8 ~ T < 240 (e4m3 inf threshold 248)


def _r(ap):
    """Reinterpret an fp32 AP as float32r for 1-cycle/row PE streaming."""
    return ap.bitcast(F32R)


N_JG = HW_N // P     # 32 key chunks of 128
N_IC = NQ // 512     # 4 query chunks of 512
N_T = N_JG // 2      # 16 tiles per chunk (2 key chunks each)

# mult-pass engine split per tile index (D=DVE, P=Pool), tuned to the
# measured 1.63us (DVE) vs 2.75us (Pool) per-tile rates.
MULT_ENG = "DDPDPDDPDPDDPDPD"


def build_program(gamma_val: float, add_bv: bool, reps: int = 1, loop_reps: int = 1):
    nc = bass.Bass("TRN2", target_bir_lowering=False, debug=False)

    x_sh = nc.dram_tensor("x_sh", [C, NQ], F32R, kind="ExternalInput").ap()
    y_sh = nc.dram_tensor("y_sh", [C, HW_N], F32R, kind="ExternalInput").ap()
    yT8_sh = nc.dram_tensor("yT8_sh", [HW_N, C], FP8, kind="ExternalInput").ap()
    wqT = nc.dram_tensor("wqT", [C, P], F32R, kind="ExternalInput").ap()
    wkT = nc.dram_tensor("wkT", [C, P], F32R, kind="ExternalInput").ap()
    wvT = nc.dram_tensor("wvT", [C, C], BF16, kind="ExternalInput").ap()
    bq = nc.dram_tensor("bq", [C8, 1], F32, kind="ExternalInput").ap()
    bk = nc.dram_tensor("bk", [C8, 1], F32, kind="ExternalInput").ap()
    bvT = nc.dram_tensor("bvT", [C, 1], F32, kind="ExternalInput").ap()
    onesg = nc.dram_tensor("onesg", [P, P], BF16, kind="ExternalInput").ap()
    out_sh = nc.dram_tensor("out_sh", [C, NQ], F32, kind="ExternalOutput").ap()

    with tile.TileContext(nc) as tc:
        from contextlib import ExitStack

        with ExitStack() as ctx:
            consts = ctx.enter_context(tc.tile_pool(name="consts", bufs=1))
            big = ctx.enter_context(tc.tile_pool(name="big", bufs=1))
            ptp = ctx.enter_context(tc.tile_pool(name="ptp", bufs=18))
            p8p = ctx.enter_context(tc.tile_pool(name="p8p", bufs=6))
            recp = ctx.enter_context(tc.tile_pool(name="recp", bufs=2))
            outp = ctx.enter_context(tc.tile_pool(name="outp", bufs=4))
            rbsp = ctx.enter_context(tc.tile_pool(name="rbsp", bufs=2))
            zsbp = ctx.enter_context(tc.tile_pool(name="zsbp", bufs=2))

            # --- constants ---
            # wqT/wkT arrive with their 32 output columns replicated 4x
            # ([C, 128]), so one M=128 matmul writes q (k) to all four
            # partition quarters of the psum at once.  wvT ([in c', out c])
            # is the stationary operand of the per-chunk out-matmuls.
            wq_sb = consts.tile([P, 2, P], F32R)
            nc.scalar.dma_start(out=wq_sb, in_=wqT.rearrange("(k p) m -> p k m", p=P))
            wk_sb = consts.tile([P, 2, P], F32R)
            nc.sync.dma_start(out=wk_sb, in_=wkT.rearrange("(k p) m -> p k m", p=P))
            wv_sb = consts.tile([P, 2, C], BF16)
            nc.sync.dma_start(out=wv_sb, in_=wvT.rearrange("(k p) m -> p k m", p=P))
            bq_sb = consts.tile([P, 1], F32)
            bk_sb = consts.tile([P, 1], F32)
            for a in range(4):
                nc.scalar.dma_start(out=bq_sb[32 * a : 32 * a + 32, :], in_=bq)
                nc.scalar.dma_start(out=bk_sb[32 * a : 32 * a + 32, :], in_=bk)
            # Stationary all-(1/T) [128, 128] bf16 block: the ones-l stream
            # replicates l/T across partitions, so reciprocal(l_ps) is
            # directly the T/l fp8-conversion scale.
            ones_col = consts.tile([P, P], BF16)
            nc.sync.dma_start(out=ones_col, in_=onesg)
            # All-ones fp8 stationary block for the l8 = sum_j p8 stream.
            ones8_sb = consts.tile([P, 2, P], FP8)
            nc.vector.memset(ones8_sb, 1.0)
            bv_sb = None
            if add_bv:
                bv_sb = consts.tile([P, 2], F32)
                nc.sync.dma_start(
                    out=bv_sb, in_=bvT.rearrange("(h p) one -> p (h one)", p=P)
                )

            # --- activations ---
            # x_sb stays fp32: the residual add must see unrounded x. Its
            # DMA writes through an fp32r view so the q-projection (which
            # reads it as fp32r) has an fp32r-typed producer.
            x_sb = big.tile([P, 2, NQ], F32)
            x_view = x_sh.rearrange("(k p) n -> p k n", p=P)
            for h in range(4):
                for kc in range(2):
                    hs = slice(h * (NQ // 4), (h + 1) * (NQ // 4))
                    nc.scalar.dma_start(out=_r(x_sb[:, kc, hs]), in_=x_view[:, kc, hs])
            # Order y chunks h-major so early slices of both C-chunks land
            # together and the k projection can start early.
            y_sb = big.tile([P, 2, HW_N], F32R)
            y_view = y_sh.rearrange("(k p) n -> p k n", p=P)
            for h in range(4):
                for kc in range(2):
                    hs = slice(h * (HW_N // 4), (h + 1) * (HW_N // 4))
                    nc.sync.dma_start(out=y_sb[:, kc, hs], in_=y_view[:, kc, hs])
            # yT8 (host-transposed y, e4m3) is the stationary operand of the
            # fp8 DoubleRow z-streams: z = yT8'-contracted-with-p8 replaces
            # the v-projection+PV pair (out = (Wv y) p == Wv (yT' p)).
            yT8_sb = big.tile([P, N_JG, C], FP8)
            yT8_view = yT8_sh.rearrange("(J p) c -> p J c", p=P)
            for Jh in range(4):
                Js = slice(Jh * 8, (Jh + 1) * 8)
                eng = nc.scalar if Jh % 2 else nc.sync
                eng.dma_start(out=yT8_sb[:, Js, :], in_=yT8_view[:, Js, :])

            # qP: q replicated in all 4 partition quarters.
            # kP[32a + c8, g*128 + jj] = k[c8, (4g + a)*128 + jj].
            qP = big.tile([P, NQ], F32R)
            kP = big.tile([P, HW_N // 4], F32R)

            def body():
                with (
                    tc.tile_pool(name="qkp", bufs=2, space="PSUM") as qkp,
                    tc.tile_pool(name="zp", bufs=1, space="PSUM") as zp,
                    tc.tile_pool(name="lp", bufs=1, space="PSUM") as lp,
                    tc.tile_pool(name="l8p", bufs=1, space="PSUM") as l8p,
                ):
                    # Projections share the chunk pipeline: a short prologue
                    # computes k blocks 0-1 and q chunk 0, and the rest rides
                    # chunk 0's per-tile filler slots (no z/mult traffic
                    # during chunk 0).  The k psum holds 4 replicas of k
                    # across partition quarters, so quarter a of key block g
                    # reads its own replica directly into the row-tiled kP
                    # layout (copies split between ScalarE and DVE).
                    def q_proj(t):
                        ps_q = zp.tile([P, 512], F32, tag="z1", name="ps_q")
                        for kc in range(2):
                            nc.tensor.matmul(
                                ps_q,
                                lhsT=wq_sb[:, kc, :],
                                rhs=_r(x_sb[:, kc, t * 512 : (t + 1) * 512]),
                                start=(kc == 0),
                                stop=(kc == 1),
                            )
                        nc.scalar.activation(
                            qP[:, t * 512 : (t + 1) * 512],
                            ps_q,
                            AF.Identity,
                            bias=bq_sb,
                        )

                    def k_proj(g):
                        ps_k = zp.tile([P, 512], F32, tag="z0", name="ps_k")
                        for kc in range(2):
                            nc.tensor.matmul(
                                ps_k,
                                lhsT=wk_sb[:, kc, :],
                                rhs=y_sb[:, kc, g * 512 : (g + 1) * 512],
                                start=(kc == 0),
                                stop=(kc == 1),
                            )
                        for a in range(4):
                            src = ps_k[32 * a : 32 * a + 32, a * P : (a + 1) * P]
                            dst = kP[32 * a : 32 * a + 32, g * P : (g + 1) * P]
                            if a < 2:
                                nc.scalar.activation(
                                    dst, src, AF.Identity,
                                    bias=bk_sb[32 * a : 32 * a + 32, :],
                                )
                            else:
                                nc.vector.tensor_scalar_add(
                                    dst, src, bk_sb[32 * a : 32 * a + 32, :]
                                )

                    def mult(st, t):
                        # fp8 conversion: p8 = pt * (T/l), split DVE/Pool by
                        # measured rates.  rect is replicated across
                        # partitions so this is a plain elementwise multiply
                        # with the scale broadcast over the two key halves.
                        p8t = p8p.tile([P, 2, 512], FP8)
                        eng = nc.vector if MULT_ENG[t] == "D" else nc.gpsimd
                        eng.tensor_mul(
                            p8t,
                            st["pts"][t].rearrange("p (u n) -> p u n", u=2),
                            st["rect"]
                            .rearrange("p (o n) -> p o n", o=1)
                            .broadcast_to((P, 2, 512)),
                        )
                        st["p8s"].append(p8t)

                    def z8_set(st, t):
                        # fp8 DoubleRow z-matmuls (both channel halves) plus
                        # the l8-denominator matmul for tile t.
                        p8t = st["p8s"][t]
                        for half, zt in ((0, st["z0"]), (1, st["z1"])):
                            nc.tensor.matmul(
                                zt,
                                lhsT=yT8_sb[:, 2 * t : 2 * t + 2,
                                            half * P : (half + 1) * P],
                                rhs=p8t,
                                start=(t == 0),
                                stop=(t == N_T - 1),
                                perf_mode=DR,
                            )
                        nc.tensor.matmul(
                            st["l8"],
                            lhsT=ones8_sb,
                            rhs=p8t,
                            start=(t == 0),
                            stop=(t == N_T - 1),
                            perf_mode=DR,
                        )

                    def z_finish(st):
                        # Drain z into SBUF right after its streams end; bf16
                        # is plenty for the pre-normalize z and makes the Wv
                        # out-matmuls stream 2 cols/cycle.
                        z_sb = zsbp.tile([P, 2, 512], BF16)
                        nc.vector.tensor_copy(z_sb[:, 0, :], st["z0"])
                        nc.vector.tensor_copy(z_sb[:, 1, :], st["z1"])
                        st["z_sb"] = z_sb

                    def emit_tail(st):
                        # Chunk tail (two fronts later): gamma/l8 multiplier,
                        # Wv out-matmuls, normalize + residual + store.  o1
                        # first: it shares its psum bank with the next
                        # chunk's z1 stream.
                        rec = rbsp.tile([P, 512], F32)
                        nc.vector.reciprocal(rec, st["l8"])
                        for cc, o in ((1, st["o1"]), (0, st["o0"])):
                            for kc in range(2):
                                nc.tensor.matmul(
                                    o,
                                    lhsT=wv_sb[:, kc, cc * P : (cc + 1) * P],
                                    rhs=st["z_sb"][:, kc, :],
                                    start=(kc == 0),
                                    stop=(kc == 1),
                                )
                            res = outp.tile([P, 512], F32)
                            # res = (o * gamma) * rec in one DVE op
                            nc.vector.scalar_tensor_tensor(
                                res, o, float(gamma_val), rec,
                                mybir.AluOpType.mult, mybir.AluOpType.mult,
                            )
                            if bv_sb is not None:
                                nc.vector.tensor_scalar_add(
                                    res, res, bv_sb[:, cc : cc + 1]
                                )
                            nc.gpsimd.tensor_add(res, res, x_sb[:, cc, st["isl"]])
                            nc.sync.dma_start(
                                out=out_sh[cc * P : (cc + 1) * P, st["isl"]], in_=res
                            )

                    def front(ic, prev, tail_st, fillers=None):
                        # QK + exp for chunk ic, with the previous chunk's
                        # fp8 conversion + z8/l8 streams and the
                        # chunk-before-previous tail interleaved.  ScalarE's
                        # exp stream is the pacer; the 2-buffer e_ps rotation
                        # keeps it fed while PE works the z-streams.
                        st = {
                            "isl": slice(ic * 512, (ic + 1) * 512),
                            "pts": [],
                            "p8s": [],
                        }
                        if prev is not None:
                            # T/l for the previous chunk -- first DVE op of
                            # this front (its l_ps completed last front).
                            rect = recp.tile([P, 512], F32)
                            nc.vector.reciprocal(rect, prev["l_ps"])
                            prev["rect"] = rect
                        st["l_ps"] = lp.tile([P, 512], F32, name="l_ps")
                        for t in range(N_T):
                            e_ps = qkp.tile([P, 1024], F32)
                            for u in range(2):
                                J = 2 * t + u
                                a, g = J % 4, J // 4
                                nc.tensor.matmul(
                                    e_ps[:, u * 512 : (u + 1) * 512],
                                    lhsT=kP[
                                        32 * a : 32 * a + 32, g * 128 : (g + 1) * 128
                                    ],
                                    rhs=qP[32 * a : 32 * a + 32, st["isl"]],
                                    start=True,
                                    stop=True,
                                    tile_position=(32 * a, 0),
                                )
                            if prev is not None:
                                mult(prev, t)
                            if t == 0 and tail_st is not None:
                                emit_tail(tail_st)
                            if prev is not None and t >= 1:
                                z8_set(prev, t - 1)
                            pt = ptp.tile([P, 1024], BF16)
                            nc.scalar.activation(pt, e_ps, AF.Exp)
                            st["pts"].append(pt)
                            # denominator: l_ps += sum_j pt (bf16 ones
                            # matmuls, 2 cols/cycle) -- for tile t-1 so the
                            # exp wait is pre-satisfied.
                            for u in range(2):
                                tt = t - 1 if t >= 1 else None
                                if tt is None:
                                    break
                                nc.tensor.matmul(
                                    st["l_ps"],
                                    lhsT=ones_col,
                                    rhs=st["pts"][tt][:, u * 512 : (u + 1) * 512],
                                    start=(tt == 0 and u == 0),
                                    stop=False,
                                )
                            if fillers:
                                fillers.pop(0)()
                        # last tile's ones-l matmuls + previous chunk's tail
                        for u in range(2):
                            nc.tensor.matmul(
                                st["l_ps"],
                                lhsT=ones_col,
                                rhs=st["pts"][N_T - 1][:, u * 512 : (u + 1) * 512],
                                start=False,
                                stop=(u == 1),
                            )
                        if prev is not None:
                            z8_set(prev, N_T - 1)
                            z_finish(prev)
                        # z/o/l8 psum tiles for THIS chunk are allocated at
                        # the end of its front: first write is one front
                        # later, keeping the one-buffer tag rotations causal
                        # (z written next front, o written the front after).
                        st["z0"] = zp.tile([P, 512], F32, tag="z0", name="z0")
                        st["o0"] = zp.tile([P, 512], F32, tag="z0", name="o0")
                        st["z1"] = zp.tile([P, 512], F32, tag="z1", name="z1")
                        st["o1"] = zp.tile([P, 512], F32, tag="z1", name="o1")
                        st["l8"] = l8p.tile([P, 512], F32, name="l8")
                        return st

                    # Prologue: just enough projection for chunk 0's first
                    # QK pairs; the rest rides chunk-0's filler slots.
                    k_proj(0)
                    q_proj(0)
                    k_proj(1)
                    fillers = [
                        lambda: k_proj(2), lambda: k_proj(3), lambda: q_proj(1),
                        lambda: k_proj(4), lambda: k_proj(5), lambda: q_proj(2),
                        lambda: k_proj(6), lambda: k_proj(7), lambda: q_proj(3),
                    ]
                    sts = []
                    for ic in range(N_IC):
                        prev = sts[ic - 1] if ic >= 1 else None
                        tail_st = sts[ic - 2] if ic >= 2 else None
                        sts.append(front(ic, prev, tail_st, fillers if ic == 0 else None))
                    # Drain: chunk 3's conversion + z-streams run solid, with
                    # chunk 2's tail before the first z8 group (bank order).
                    st3 = sts[3]
                    rect = recp.tile([P, 512], F32)
                    nc.vector.reciprocal(rect, st3["l_ps"])
                    st3["rect"] = rect
                    mult(st3, 0)
                    emit_tail(sts[2])
                    for t in range(N_T):
                        if t + 1 < N_T:
                            mult(st3, t + 1)
                        z8_set(st3, t)
                    z_finish(st3)
                    emit_tail(st3)

            if loop_reps > 1:
                with tc.For_i(0, loop_reps, 1):
                    body()
            else:
                for _ in range(reps):
                    body()

    return nc


def kernel(x, y, Wq, bq, Wk, bk, Wv, bv, gamma):
    x = np.ascontiguousarray(np.asarray(x, dtype=np.float32))
    y = np.ascontiguousarray(np.asarray(y, dtype=np.float32))
    gamma_val = float(np.asarray(gamma).reshape(-1)[0])
    bv_arr = np.asarray(bv, dtype=np.float32).reshape(1, C)
    add_bv = bool(np.any(bv_arr))

    nc = build_program(gamma_val, add_bv)

    res = run_bass_kernel_spmd(
        nc,
        make_in_maps(x, y, Wq, bq, Wk, bk, Wv, bv, gamma_val),
        core_ids=list(range(NCORES)),
    )

    out = np.empty((B, C, HW_N), dtype=np.float32)
    for core in range(NCORES):
        b, h = core // 2, core % 2
        out[b][:, h * NQ : (h + 1) * NQ] = res.results[core]["out_sh"]
    return out.reshape(B, C, 64, 64)


def make_in_maps(x, y, Wq, bq, Wk, bk, Wv, bv, gamma_val=0.5):
    xf = np.asarray(x, dtype=np.float32).reshape(B, C, HW_N)
    yf = np.asarray(y, dtype=np.float32).reshape(B, C, HW_N)
    wqT = np.ascontiguousarray(np.tile(np.asarray(Wq, dtype=np.float32).T, (1, 4)))
    wkT = np.ascontiguousarray(np.tile(np.asarray(Wk, dtype=np.float32).T, (1, 4)))
    wvT = np.ascontiguousarray(np.asarray(Wv, dtype=np.float32).T.astype(ml_dtypes.bfloat16))
    bq_arr = np.asarray(bq, dtype=np.float32).reshape(C8, 1)
    bk_arr = np.asarray(bk, dtype=np.float32).reshape(C8, 1)
    # bv rides the tail as res += gamma*bv (sum_j softmax == 1).
    bvT_arr = (gamma_val * np.asarray(bv, dtype=np.float32)).reshape(C, 1)
    onesg = np.full((P, P), 1.0 / T_SCALE, dtype=ml_dtypes.bfloat16)

    in_maps = []
    for core in range(NCORES):
        b, h = core // 2, core % 2
        in_maps.append(
            {
                "x_sh": np.ascontiguousarray(xf[b][:, h * NQ : (h + 1) * NQ]),
                "y_sh": np.ascontiguousarray(yf[b]),
                "yT8_sh": np.ascontiguousarray(
                    yf[b].T.astype(ml_dtypes.float8_e4m3)
                ),
                "wqT": wqT,
                "wkT": wkT,
                "wvT": wvT,
                "bq": bq_arr,
                "bk": bk_arr,
                "bvT": bvT_arr,
                "onesg": onesg,
            }
        )
    return in_maps
